# revision 1
# baseline (speedup 1.0000x reference)
"""ConvLSTM latent-cell kernel for 8x Trainium2 NeuronCores.

Model (matches reference):
  x = x_audio + pe(midi_notes)                       [B,T,16,64,64]
  layer0 = bidirectional ConvLSTM(16 -> 32), 3x3 SAME
  layer1 = bidirectional ConvLSTM(64 -> 32), 3x3 SAME
  out    = conv3x3(layer1_out[T-1], 64 -> 64) + bias [B,64,64,64]

Sharding: data-parallel over batch, B=16 -> 2 images per core, weights
replicated, no collectives.  Only h[-1] of layer 1 is consumed, so the
layer-1 backward scan runs a single step (its t=T-1 output is step 0 of
the reversed scan).

Per-core program: channels on the matmul contraction dim; conv = 9
shifted matmuls (full M=128 output channels) accumulating in PSUM from
a zero-padded [Cin, 66*66] bf16 input buffer.  Gate nonlinearities run
on ScalarE while evacuating PSUM; gate tensors are then re-tiled via
SBUF->SBUF DMAs into a "folded" [128, 1024] layout (4 spatial blocks x
32 gate channels) so the LSTM cell elementwise math uses all 128
partitions at base partition 0 (VectorE requires equal input base
partitions).  Cell state and gate math stay fp32; bf16 is used only for
matmul inputs and DMA staging of h.
"""

import numpy as np
import ml_dtypes

# Model dims (fixed by the problem)
B, T, C, H, W = 16, 8, 16, 64, 64
HID, LAT, KS = 32, 64, 3
NCORES = 8
BL = B // NCORES           # images per core

bf16 = ml_dtypes.bfloat16


# ----------------------------------------------------------------------------
# Host-side preprocessing
# ----------------------------------------------------------------------------

def _note_encoder_pe(midi_notes, enc_w1, enc_b1, enc_w2, enc_b2):
    # matches reference: tanh(relu((-0.5 + m/64) @ w1 + b1) @ w2 + b2)
    notes = (-0.5 + midi_notes / np.float32(64.0)).astype(np.float32)
    a = np.maximum(notes @ enc_w1 + enc_b1, 0.0).astype(np.float32)
    pe = np.tanh(a @ enc_w2 + enc_b2).astype(np.float32)
    return pe  # [B, T, C*H*W]


def _paired_weights(w, row_perm, kpair):
    # stack taps (dy,0) and (dy,2): rows [0:cin]=W(dy,0), pad, [kpair-cin:]=W(dy,2)
    cout, cin = w.shape[0], w.shape[1]
    wt = w.transpose(1, 2, 3, 0)          # [cin, 3, 3, cout]
    if row_perm is not None:
        wt = wt[row_perm]
    out = np.zeros((kpair, 3 * cout), np.float32)
    for dy in range(3):
        out[0:cin, dy * cout:(dy + 1) * cout] = wt[:, dy, 0, :]
        out[kpair - cin:, dy * cout:(dy + 1) * cout] = wt[:, dy, 2, :]
    return out.astype(bf16)


def _weights_to_sb(w, row_perm=None):
    # w: [Cout, Cin, 3, 3] -> [Cin, 9*Cout], col = (dy*3+dx)*Cout + co
    cout, cin = w.shape[0], w.shape[1]
    ws = w.transpose(1, 2, 3, 0).reshape(cin, 9 * cout)
    if row_perm is not None:
        ws = ws[row_perm]
    return np.ascontiguousarray(ws).astype(bf16)


# ----------------------------------------------------------------------------
# Device program
# ----------------------------------------------------------------------------

_PROGRAM_CACHE = {}


def _build_program(bl=BL, t_steps=T, hh=H, ww=W, repeats=1):
    """Emit the per-core Bass/Tile program.  Returns nc."""
    import concourse.bass as bass
    import concourse.tile as tile
    from concourse import bacc, mybir

    f32 = mybir.dt.float32
    b16 = mybir.dt.bfloat16
    AF = mybir.ActivationFunctionType

    hp = hh + 2
    padn = hp * hp
    hw = hh * ww
    FOLD = 4
    fw = hw // FOLD                 # free size of a folded tile
    rows_per_block = hh // FOLD     # spatial rows per fold block
    NCH = 8                         # matmul free-dim chunks (<=512 each)
    chunk = hw // NCH
    rows_per_chunk = hh // NCH
    half_hw = hw // 2

    nc = bacc.Bacc("TRN2", target_bir_lowering=False, debug=False, num_devices=1)

    xa = nc.dram_tensor("xa", [bl, t_steps, C, hw], b16, kind="ExternalInput")
    w0f = nc.dram_tensor("w0f", [C + HID, 9 * 4 * HID], b16, kind="ExternalInput")
    w0b = nc.dram_tensor("w0b", [C + HID, 9 * 4 * HID], b16, kind="ExternalInput")
    w1f = nc.dram_tensor("w1f", [3 * HID, 9 * 4 * HID], b16, kind="ExternalInput")
    w1b = nc.dram_tensor("w1b", [3 * HID, 9 * 4 * HID], b16, kind="ExternalInput")
    wfc = nc.dram_tensor("wfc", [2 * HID, 9 * LAT], b16, kind="ExternalInput")
    w0fp = nc.dram_tensor("w0fp", [112, 3 * 4 * HID], b16, kind="ExternalInput")
    w0bp = nc.dram_tensor("w0bp", [112, 3 * 4 * HID], b16, kind="ExternalInput")
    wfcp = nc.dram_tensor("wfcp", [128, 3 * LAT], b16, kind="ExternalInput")
    biases = nc.dram_tensor("biases", [128, 5], f32, kind="ExternalInput")
    out = nc.dram_tensor("out", [bl, LAT, hw], f32, kind="ExternalOutput")
    h0d = nc.dram_tensor("h0d", [bl, t_steps, 2 * HID, hw], b16)

    with tile.TileContext(nc) as tc:
        import contextlib
        with contextlib.ExitStack() as ctx:
            persist = ctx.enter_context(tc.tile_pool(name="persist", bufs=1))
            gates_pool = ctx.enter_context(tc.tile_pool(name="gates", bufs=2))
            psum_pool = ctx.enter_context(
                tc.tile_pool(name="ps", bufs=2, space="PSUM")
            )
            hout_pool = ctx.enter_context(tc.tile_pool(name="hout", bufs=2))

            # ---- persistent tiles -------------------------------------------
            w0f_sb = persist.tile([C + HID, 9 * 4 * HID], b16, tag="w0f")
            w0b_sb = persist.tile([C + HID, 9 * 4 * HID], b16, tag="w0b")
            w1f_sb = persist.tile([3 * HID, 9 * 4 * HID], b16, tag="w1f")
            w1b_sb = persist.tile([3 * HID, 9 * 4 * HID], b16, tag="w1b")
            wfc_sb = persist.tile([2 * HID, 9 * LAT], b16, tag="wfc")
            w0fp_sb = persist.tile([112, 3 * 4 * HID], b16, tag="w0fp")
            w0bp_sb = persist.tile([112, 3 * 4 * HID], b16, tag="w0bp")
            wfcp_sb = persist.tile([128, 3 * LAT], b16, tag="wfcp")
            bias_sb = persist.tile([128, 5], f32, tag="bias")
            nc.sync.dma_start(w0f_sb[:], w0f[:])
            nc.sync.dma_start(w0b_sb[:], w0b[:])
            nc.sync.dma_start(w1f_sb[:], w1f[:])
            nc.sync.dma_start(w1b_sb[:], w1b[:])
            nc.sync.dma_start(wfc_sb[:], wfc[:])
            nc.sync.dma_start(w0fp_sb[:], w0fp[:])
            nc.sync.dma_start(w0bp_sb[:], w0bp[:])
            nc.sync.dma_start(wfcp_sb[:], wfcp[:])
            nc.sync.dma_start(bias_sb[:], biases[:])

            # conv input buffers (zero-padded), one per image+layer
            l0buf = [persist.tile([112, padn], b16, tag=f"l0b{i}",
                                  name=f"l0b{i}") for i in range(bl)]
            l1buf = [persist.tile([3 * HID, padn], b16, tag=f"l1b{i}",
                                  name=f"l1b{i}") for i in range(bl)]
            fcbuf = [persist.tile([128, padn], b16, tag=f"fcb{i}",
                                  name=f"fcb{i}") for i in range(bl)]
            cstate = [persist.tile([128, fw], f32, tag=f"c{i}", name=f"c{i}")
                      for i in range(bl)]

            for i in range(bl):
                nc.vector.memset(l0buf[i][:], 0.0)
                nc.vector.memset(l1buf[i][:], 0.0)
                nc.vector.memset(fcbuf[i][:], 0.0)

            # ---- helpers ----------------------------------------------------
            _dma_rr = [0]

            def dma(dst, src):
                eng = nc.sync if _dma_rr[0] % 2 == 0 else nc.gpsimd
                _dma_rr[0] += 1
                eng.dma_start(dst, src)

            def rhs_slice(buf, cin, tap, j):
                dy, dx = tap // 3, tap % 3
                r0 = rows_per_chunk * j + dy
                return (buf[0:cin, :]
                        .rearrange("p (r c) -> p r c", c=hp)
                        [:, r0:r0 + rows_per_chunk, dx:dx + ww])

            def interior(buf, pbase, pcnt, a, shift=0):
                r0, nr = rows_per_block * a, rows_per_block
                off = (r0 + 1) * hp + 1 - shift
                return (buf[pbase:pbase + pcnt, off:off + nr * hp]
                        .rearrange("p (r c) -> p r c", c=hp)[:, :, 0:ww])

            def conv_z_to_S(buf, cin, w_sb, bias_col, S, mparts=128,
                            act=None, Fg=None, pair=None):
                """z = conv(buf) for all output channels; nonlinearity applied
                while evacuating PSUM into S [mparts, hw]."""
                for hlf in range(2):
                    ps = psum_pool.tile([128, half_hw], f32, tag="ps",
                                        name="ps")
                    for jj in range(NCH // 2):
                        j = hlf * (NCH // 2) + jj
                        if pair is None:
                            for tap in range(9):
                                nc.tensor.matmul(
                                    ps[0:mparts,
                                       chunk * jj:chunk * (jj + 1)],
                                    w_sb[0:cin,
                                         tap * mparts:(tap + 1) * mparts],
                                    rhs_slice(buf, cin, tap, j),
                                    start=(tap == 0), stop=(tap == 8),
                                )
                        else:
                            wp_sb, kpair = pair
                            for sweep in range(6):
                                dy = sweep % 3
                                if sweep < 3:   # paired (dy,0)+(dy,2)
                                    lhsT = wp_sb[0:kpair,
                                                 dy * mparts:(dy + 1) * mparts]
                                    rhs = rhs_slice(buf, kpair, 3 * dy, j)
                                else:           # single (dy,1)
                                    lhsT = w_sb[0:cin,
                                                (3 * dy + 1) * mparts:
                                                (3 * dy + 2) * mparts]
                                    rhs = rhs_slice(buf, cin, 3 * dy + 1, j)
                                nc.tensor.matmul(
                                    ps[0:mparts,
                                       chunk * jj:chunk * (jj + 1)],
                                    lhsT, rhs,
                                    start=(sweep == 0), stop=(sweep == 5),
                                )
                    cols = slice(half_hw * hlf, half_hw * (hlf + 1))
                    if act == "gates":
                        nc.scalar.activation(
                            S[0:96, cols], ps[0:96, :], AF.Sigmoid,
                            bias=bias_sb[0:96, bias_col:bias_col + 1])
                        for jj in range(NCH // 2):
                            j = hlf * (NCH // 2) + jj
                            a, wh = j // 2, j % 2
                            nc.scalar.activation(
                                Fg[32 * a:32 * a + 32,
                                   chunk * wh:chunk * (wh + 1)],
                                ps[96:128, chunk * jj:chunk * (jj + 1)],
                                AF.Tanh,
                                bias=bias_sb[96:128, bias_col:bias_col + 1])
                    else:
                        nc.scalar.activation(
                            S[0:mparts, cols], ps[0:mparts, :], AF.Identity,
                            bias=bias_sb[0:mparts, bias_col:bias_col + 1])

            def fold(S, q, name):
                Ft = gates_pool.tile([128, fw], f32, tag=name, name=name)
                for a in range(FOLD):
                    dma(Ft[32 * a:32 * a + 32, :],
                        S[32 * q:32 * q + 32, fw * a:fw * (a + 1)])
                return Ft

            def lstm_step(img, buf, cin, w_sb, bias_col, first,
                          h_targets, h0_store, pair=None):
                """One ConvLSTM cell step (gate channel order i,f,o,g)."""
                c = cstate[img]
                S = gates_pool.tile([96, hw], f32, tag="S", name="S")
                Fg = gates_pool.tile([128, fw], f32, tag="Fg", name="Fg")
                conv_z_to_S(buf, cin, w_sb, bias_col, S, act="gates", Fg=Fg,
                            pair=pair)

                Fi = fold(S, 0, "Fi")
                if first:
                    # c = sigmoid(i) * tanh(g)
                    nc.vector.tensor_mul(c[:], Fi[:], Fg[:])
                else:
                    Ff = fold(S, 1, "Ff")
                    t1 = gates_pool.tile([128, fw], f32, tag="t1", name="t1")
                    nc.vector.tensor_mul(t1[:], Fi[:], Fg[:])
                    t2 = gates_pool.tile([128, fw], f32, tag="t2", name="t2")
                    nc.vector.tensor_mul(t2[:], Ff[:], c[:])
                    nc.vector.tensor_add(c[:], t1[:], t2[:])
                Fo = fold(S, 2, "Fo")

                tc_sb = gates_pool.tile([128, fw], f32, tag="tc_sb",
                                        name="tc_sb")
                nc.scalar.activation(tc_sb[:], c[:], AF.Tanh)

                h2 = hout_pool.tile([128, fw], b16, tag="h2", name="h2")
                nc.vector.tensor_mul(h2[:], Fo[:], tc_sb[:])

                for tbuf, pbase, shifted in h_targets:
                    for a in range(FOLD):
                        dma(interior(tbuf, pbase, 32, a),
                            h2[32 * a:32 * a + 32, :])
                        if shifted is not None:
                            dma(interior(tbuf, shifted, 32, a, shift=2),
                                h2[32 * a:32 * a + 32, :])
                if h0_store is not None:
                    for a in range(FOLD):
                        dma(h0_store[:, fw * a:fw * (a + 1)],
                            h2[32 * a:32 * a + 32, :])

            def load_x_l0(img, t, buf):
                # xa[img, t] -> x region (parts HID:HID+C) interior,
                # plus the +2-shifted copy at parts 96:112
                half = hh // 2
                for r in range(2):
                    src = xa[img, t, :, r * half * ww:(r + 1) * half * ww]
                    off = (r * half + 1) * hp + 1
                    dst = (buf[HID:HID + C, off:off + half * hp]
                           .rearrange("p (r c) -> p r c", c=hp)[:, :, 0:ww])
                    dma(dst, src)
                    dst2 = (buf[96:96 + C, off - 2:off - 2 + half * hp]
                            .rearrange("p (r c) -> p r c", c=hp)[:, :, 0:ww])
                    dma(dst2, src)

            def load_x_l1(img, t, buf):
                # h0d[img, t] (64 ch) -> parts 0:64 interior
                half = hh // 2
                for d in range(2):
                    for r in range(2):
                        src = h0d[img, t, d * HID:(d + 1) * HID,
                                  r * half * ww:(r + 1) * half * ww]
                        off = (r * half + 1) * hp + 1
                        dst = (buf[d * HID:(d + 1) * HID, off:off + half * hp]
                               .rearrange("p (r c) -> p r c", c=hp)
                               [:, :, 0:ww])
                        dma(dst, src)

            # ---- the schedule ----------------------------------------------
            for _rep in range(repeats):
                # layer 0, forward then backward
                for direction in ("f", "b"):
                    w_sb = {"f": w0f_sb, "b": w0b_sb}[direction]
                    bias_col = {"f": 0, "b": 1}[direction]
                    dirb = {"f": 0, "b": 1}[direction]
                    wp_sb = {"f": w0fp_sb, "b": w0bp_sb}[direction]
                    for img in range(bl):
                        nc.vector.memset(cstate[img][:], 0.0)
                        nc.vector.memset(l0buf[img][0:HID, :], 0.0)
                        nc.vector.memset(l0buf[img][64:64 + HID, :], 0.0)
                        load_x_l0(img, t_steps - 1 if direction == "b" else 0,
                                  l0buf[img])
                    for k in range(t_steps):
                        t = (t_steps - 1 - k) if direction == "b" else k
                        t_nxt = (t - 1) if direction == "b" else (t + 1)
                        for img in range(bl):
                            lstm_step(
                                img, l0buf[img], C + HID, w_sb, bias_col,
                                first=(k == 0),
                                h_targets=([(l0buf[img], 0, 64)]
                                           if k + 1 < t_steps else []),
                                h0_store=h0d[img, t,
                                             dirb * HID:(dirb + 1) * HID, :],
                                pair=(wp_sb, 112),
                            )
                            if k + 1 < t_steps:
                                load_x_l0(img, t_nxt, l0buf[img])

                # layer 1 forward
                for img in range(bl):
                    nc.vector.memset(cstate[img][:], 0.0)
                    nc.vector.memset(l1buf[img][2 * HID:3 * HID, :], 0.0)
                    load_x_l1(img, 0, l1buf[img])
                for k in range(t_steps):
                    for img in range(bl):
                        last = (k == t_steps - 1)
                        lstm_step(
                            img, l1buf[img], 3 * HID, w1f_sb, 2,
                            first=(k == 0),
                            h_targets=([(fcbuf[img], 0, 64)] if last
                                       else [(l1buf[img], 2 * HID, None)]),
                            h0_store=None,
                        )
                        if not last:
                            load_x_l1(img, k + 1, l1buf[img])

                # layer 1 backward: single step on x = h0[T-1], h = c = 0
                for img in range(bl):
                    nc.vector.memset(cstate[img][:], 0.0)
                    nc.vector.memset(l1buf[img][2 * HID:3 * HID, :], 0.0)
                    load_x_l1(img, t_steps - 1, l1buf[img])
                    lstm_step(
                        img, l1buf[img], 3 * HID, w1b_sb, 3, first=True,
                        h_targets=[(fcbuf[img], HID, 96)], h0_store=None,
                    )

                # final conv: fcbuf (64ch) -> out (64ch) + bias
                for img in range(bl):
                    Z = gates_pool.tile([64, hw], f32, tag="Z", name="Z")
                    conv_z_to_S(fcbuf[img], 2 * HID, wfc_sb, 4, Z,
                                mparts=64, act=None, pair=(wfcp_sb, 128))
                    for hlf in range(2):
                        nc.sync.dma_start(
                            out[img, :, half_hw * hlf:half_hw * (hlf + 1)],
                            Z[0:64, half_hw * hlf:half_hw * (hlf + 1)])

    nc.compile()
    return nc


def _get_program(key=(BL, T, H, W)):
    if key not in _PROGRAM_CACHE:
        _PROGRAM_CACHE[key] = _build_program(*key)
    return _PROGRAM_CACHE[key]


# ----------------------------------------------------------------------------
# Entry point
# ----------------------------------------------------------------------------

def _prep_in_maps(x_audio, midi_notes, enc_w1, enc_b1, enc_w2, enc_b2,
                  w0f, b0f, w0b, b0b, w1f, b1f, w1b, b1b, fc_w, fc_b,
                  n_cores=NCORES):
    bsz = x_audio.shape[0]
    bl = bsz // n_cores
    pe = _note_encoder_pe(midi_notes, enc_w1, enc_b1, enc_w2, enc_b2)
    x = (np.asarray(x_audio, np.float32)
         + pe.reshape(x_audio.shape)).astype(bf16)
    x = x.reshape(bsz, T, C, H * W)

    perm0 = list(range(C, C + HID)) + list(range(C))
    w0f_t, w0b_t = _weights_to_sb(w0f, perm0), _weights_to_sb(w0b, perm0)
    w1f_t, w1b_t = _weights_to_sb(w1f), _weights_to_sb(w1b)
    wfc_t = _weights_to_sb(fc_w)

    w0fp_t = _paired_weights(w0f, perm0, 112)
    w0bp_t = _paired_weights(w0b, perm0, 112)
    wfcp_t = _paired_weights(fc_w, None, 128)

    bias_cols = []
    for b in (b0f, b0b, b1f, b1b):
        bias_cols.append(np.asarray(b, np.float32))
    bias_cols.append(np.pad(np.asarray(fc_b, np.float32), (0, 64)))
    biases = np.stack(bias_cols, axis=1).astype(np.float32)  # [128, 5]

    in_maps = []
    for core in range(n_cores):
        sl = slice(core * bl, (core + 1) * bl)
        in_maps.append({
            "xa": np.ascontiguousarray(x[sl]),
            "w0f": w0f_t, "w0b": w0b_t, "w1f": w1f_t, "w1b": w1b_t,
            "wfc": wfc_t, "w0fp": w0fp_t, "w0bp": w0bp_t, "wfcp": wfcp_t,
            "biases": biases,
        })
    return in_maps


def kernel(x_audio, midi_notes, enc_w1, enc_b1, enc_w2, enc_b2,
           w0f, b0f, w0b, b0b, w1f, b1f, w1b, b1b, fc_w, fc_b):
    from concourse.bass_utils import run_bass_kernel_spmd

    nc = _get_program()
    in_maps = _prep_in_maps(
        x_audio, midi_notes, enc_w1, enc_b1, enc_w2, enc_b2,
        w0f, b0f, w0b, b0b, w1f, b1f, w1b, b1b, fc_w, fc_b,
    )
    res = run_bass_kernel_spmd(nc, in_maps, list(range(NCORES)))
    outs = [r["out"].reshape(BL, LAT, H, W) for r in res.results]
    return np.concatenate(outs, axis=0).astype(np.float32)



# revision 25
# speedup vs baseline: 1.2833x; 1.2833x over previous
"""ConvLSTM latent-cell kernel for 8x Trainium2 NeuronCores.

Model (matches reference):
  x = x_audio + pe(midi_notes)                       [B,T,16,64,64]
  layer0 = bidirectional ConvLSTM(16 -> 32), 3x3 SAME
  layer1 = bidirectional ConvLSTM(64 -> 32), 3x3 SAME
  out    = conv3x3(layer1_out[T-1], 64 -> 64) + bias [B,64,64,64]

Sharding: data-parallel over batch, B=16 -> 2 images per core, weights
replicated, no collectives.  Only h[-1] of layer 1 is consumed, so the
layer-1 backward scan runs a single step.

Per-core program highlights:
  * conv = shifted matmuls over a zero-padded [*, 66*66] bf16 buffer.
    Layer 0 packs (channel, tap) pairs into 4 passes/step (3x K=128 +
    1x K=48) using column/row-shifted input copies; layer 1 runs the
    plain 9 passes of K=96.
  * all 4 gates are evacuated from PSUM by a single wide sigmoid
    activation per PSUM half using tanh(x) = 2*sigmoid(2x) - 1 with a
    per-partition scale vector (2 on the g rows); h is kept at HALF
    scale ((sig(2c)-0.5)*sig(o)) which is compensated by doubling the
    h-input rows of downstream conv weights.
  * gate re-tiling ("fold") into the [128 = 4 spatial blocks x 32ch,
    1024] layout is ONE dma per PSUM half via partition<->free
    rearrange; h write-back / h0 store / x loads are one dma each.
  * forward and backward layer-0 streams are interleaved (4 independent
    recurrences) to keep the PE warm and hide the per-step gate tail.
"""

import numpy as np
import ml_dtypes

B, T, C, H, W = 16, 8, 16, 64, 64
HID, LAT, KS = 32, 64, 3
NCORES = 8
BL = B // NCORES

hp = H + 2                 # padded row width
padn = hp * hp             # padded image size
hw = H * W
FOLD = 4
half_hw = hw // 2
NCH = 8                    # matmul free-dim chunks over hw (512 each)
chunk = hw // NCH
rows_per_chunk = H // NCH
# folded tiles keep a 66-column row pitch (2 "junk" columns per 64-pixel
# row that compute to exactly 0 through the gate chain) so that the
# h write-back into the padded conv buffers is a single contiguous-span
# DMA per shifted copy.
FW = 16 * hp               # 1056: one fold block (16 rows x 66)
FT = FOLD * FW             # 4224: full folded free size

f16 = np.float16


# ----------------------------------------------------------------------------
# Host-side preprocessing
# ----------------------------------------------------------------------------

def _note_encoder_pe(midi_notes, enc_w1, enc_b1, enc_w2, enc_b2):
    notes = (-0.5 + midi_notes / np.float32(64.0)).astype(np.float32)
    a = np.maximum(notes @ enc_w1 + enc_b1, 0.0).astype(np.float32)
    pe = np.tanh(a @ enc_w2 + enc_b2).astype(np.float32)
    return pe  # [B, T, C*H*W]


def _pad_shift(x, dy, dx):
    """x: [N, ch, H, W] -> padded [N, ch, hp, hp] such that padded (r, c)
    holds x[r-1+dy, c-1+dx] (zero outside)."""
    n, ch = x.shape[0], x.shape[1]
    out = np.zeros((n, ch, hp, hp), np.float32)
    r0, r1 = max(0, 1 - dy), min(hp, H + 1 - dy)
    c0, c1 = max(0, 1 - dx), min(hp, W + 1 - dx)
    out[:, :, r0:r1, c0:c1] = x[:, :, r0 - 1 + dy:r1 - 1 + dy,
                                c0 - 1 + dx:c1 - 1 + dx]
    return out


def _pack_l0_weights(w):
    """w: [128, 48, 3, 3] with cin = [x(16); h(32)].
    Returns WX [128, 3*128], WY [48, 128].
    X partition layout: [h@(0,0); h@(0,1); h@(0,2); x@(0,0); x@(0,1)].
    h rows are scaled 2x (h is stored at half scale)."""
    wx = w[:, 0:C]          # [128, 16, 3, 3]
    wh = w[:, C:] * 2.0     # [128, 32, 3, 3]
    WX = np.zeros((3, 128, 128), np.float32)
    for p in range(3):
        WX[p, 0:32] = wh[:, :, p, 0].T
        WX[p, 32:64] = wh[:, :, p, 1].T
        WX[p, 64:96] = wh[:, :, p, 2].T
        WX[p, 96:112] = wx[:, :, p, 0].T
        WX[p, 112:128] = wx[:, :, p, 1].T
    WY = np.zeros((48, 128), np.float32)
    for p in range(3):
        WY[16 * p:16 * (p + 1)] = wx[:, :, p, 2].T
    return (WX.transpose(1, 0, 2).reshape(128, 3 * 128).astype(f16),
            WY.astype(f16))


def _pack_l1_weights(w):
    """w: [128, 96, 3, 3], cin = [h0f; h0b; h1] (all half-scale -> 2x).
    Returns [96, 9*128], col block (3*dy+dx)."""
    ws = (w * 2.0).transpose(1, 2, 3, 0).reshape(96, 9 * 128)
    return np.ascontiguousarray(ws).astype(f16)


def _pack_fc_weights(w):
    """w: [64, 64, 3, 3], cin = [h1f(32); h1b(32)] half-scale -> 2x.
    fcX layout [h1f@(0,0); h1b@(0,0); h1f@(0,1); h1b@(0,1)],
    fcY layout [h1f@(0,2); h1b@(0,2)].
    Returns WX [128, 3*64], WY [64, 3*64]."""
    w2 = w * 2.0
    WX = np.zeros((3, 128, 64), np.float32)
    WY = np.zeros((3, 64, 64), np.float32)
    for p in range(3):
        WX[p, 0:64] = w2[:, :, p, 0].T
        WX[p, 64:128] = w2[:, :, p, 1].T
        WY[p] = w2[:, :, p, 2].T
    return (WX.transpose(1, 0, 2).reshape(128, 3 * 64).astype(f16),
            WY.transpose(1, 0, 2).reshape(64, 3 * 64).astype(f16))


# ----------------------------------------------------------------------------
# Device program
# ----------------------------------------------------------------------------

_PROGRAM_CACHE = {}


def _build_program(bl=BL, t_steps=T, debug_taps=False):
    import contextlib
    import concourse.bass as bass
    import concourse.tile as tile
    from concourse import bacc, mybir

    f32 = mybir.dt.float32
    f16d = mybir.dt.float16
    AF = mybir.ActivationFunctionType
    OP = mybir.AluOpType

    nc = bacc.Bacc("TRN2", target_bir_lowering=False, debug=False,
                   num_devices=1)

    xpX = nc.dram_tensor("xpX", [bl, t_steps, 32, padn], f16d,
                         kind="ExternalInput")
    xpY = nc.dram_tensor("xpY", [bl, t_steps, 48, padn], f16d,
                         kind="ExternalInput")
    wX0f = nc.dram_tensor("wX0f", [128, 384], f16d, kind="ExternalInput")
    wY0f = nc.dram_tensor("wY0f", [48, 128], f16d, kind="ExternalInput")
    wX0b = nc.dram_tensor("wX0b", [128, 384], f16d, kind="ExternalInput")
    wY0b = nc.dram_tensor("wY0b", [48, 128], f16d, kind="ExternalInput")
    w1f = nc.dram_tensor("w1f", [96, 1152], f16d, kind="ExternalInput")
    w1b = nc.dram_tensor("w1b", [96, 1152], f16d, kind="ExternalInput")
    wfX = nc.dram_tensor("wfX", [128, 192], f16d, kind="ExternalInput")
    wfY = nc.dram_tensor("wfY", [64, 192], f16d, kind="ExternalInput")
    biases = nc.dram_tensor("biases", [128, 6], f32, kind="ExternalInput")
    out = nc.dram_tensor("out", [bl, LAT, hw], f32, kind="ExternalOutput")
    if debug_taps:
        h0d = nc.dram_tensor("h0d", [bl, t_steps, 2 * HID, FT], f16d,
                             kind="ExternalOutput")
    else:
        h0d = nc.dram_tensor("h0d", [bl, t_steps, 2 * HID, FT], f16d)
    h1d = nc.dram_tensor("h1d", [bl, 32, FT], f16d)
    fcd = nc.dram_tensor("fcd", [bl, 2, 32, FT], f16d)
    if debug_taps:
        dbgS = nc.dram_tensor("dbgS", [128, hw], f16d, kind="ExternalOutput")
        dbgF = nc.dram_tensor("dbgF", [128, FT], f16d, kind="ExternalOutput")
        dbgh = nc.dram_tensor("dbgh", [128, FW], f16d, kind="ExternalOutput")

    with tile.TileContext(nc) as tc:
        with contextlib.ExitStack() as ctx:
            persist = ctx.enter_context(tc.tile_pool(name="persist", bufs=1))
            spool = ctx.enter_context(tc.tile_pool(name="spool", bufs=2))
            fpool = ctx.enter_context(tc.tile_pool(name="fpool", bufs=2))
            vpool = ctx.enter_context(tc.tile_pool(name="vpool", bufs=2))
            hpool = ctx.enter_context(tc.tile_pool(name="hpool", bufs=3))
            psum_pool = ctx.enter_context(
                tc.tile_pool(name="ps", bufs=2, space="PSUM"))

            # ---- persistent tiles ------------------------------------------
            wX0f_sb = persist.tile([128, 384], f16d, tag="wX0f")
            wY0f_sb = persist.tile([48, 128], f16d, tag="wY0f")
            wX0b_sb = persist.tile([128, 384], f16d, tag="wX0b")
            wY0b_sb = persist.tile([48, 128], f16d, tag="wY0b")
            w1f_sb = persist.tile([96, 1152], f16d, tag="w1f")
            w1b_sb = persist.tile([96, 1152], f16d, tag="w1b")
            wfX_sb = persist.tile([128, 192], f16d, tag="wfX")
            wfY_sb = persist.tile([64, 192], f16d, tag="wfY")
            bias_sb = persist.tile([128, 6], f32, tag="bias")
            for sb, dr in ((wX0f_sb, wX0f), (wY0f_sb, wY0f),
                           (wX0b_sb, wX0b), (wY0b_sb, wY0b),
                           (w1f_sb, w1f), (w1b_sb, w1b),
                           (wfX_sb, wfX), (wfY_sb, wfY), (bias_sb, biases)):
                nc.sync.dma_start(sb[:], dr[:])

            # conv input buffers: per (dir, img) for L0; per img for L1/FC
            Xb = {}
            Yb = {}
            for d in range(2):
                for i in range(bl):
                    Xb[d, i] = persist.tile([128, padn], f16d,
                                            tag=f"X{d}{i}", name=f"X{d}{i}")
                    Yb[d, i] = persist.tile([48, padn], f16d,
                                            tag=f"Y{d}{i}", name=f"Y{d}{i}")
            L1b = [persist.tile([96, padn], f16d, tag=f"L1b{i}",
                                name=f"L1b{i}") for i in range(bl)]
            # fc conv buffers reuse the (done-by-then) layer-0 X tiles;
            # they are re-zeroed after layer 0 finishes.
            fcX = [Xb[0, i] for i in range(bl)]
            fcY = [Xb[1, i] for i in range(bl)]
            # c states: 4 L0 streams; L1 reuses the d=0 tiles
            cst = {}
            for d in range(2):
                for i in range(bl):
                    cst["l0", d, i] = persist.tile(
                        [128, FW], f32, tag=f"c0{d}{i}", name=f"c0{d}{i}")
            for i in range(bl):
                cst["l1", i] = cst["l0", 0, i]

            # memsets (pads; h rows).  x rows of X / all of Y are fully
            # rewritten by the padded host images every step.
            for d in range(2):
                for i in range(bl):
                    nc.vector.memset(Xb[d, i][0:96, :], 0.0)
            for i in range(bl):
                nc.gpsimd.memset(L1b[i][:], 0.0)
                nc.gpsimd.memset(fcX[i][:], 0.0)
                nc.gpsimd.memset(fcY[i][:], 0.0)

            # ---- helpers ---------------------------------------------------
            def rhs_slice(buf, cin, dy, dx, j):
                r0 = rows_per_chunk * j + dy
                return (buf[0:cin, :]
                        .rearrange("p (r c) -> p r c", c=hp)
                        [:, r0:r0 + rows_per_chunk, dx:dx + W])

            def h_dst(buf, pbase, shift):
                start = (hp + 1) - shift
                return buf[pbase:pbase + 32, start:start + FT]

            def h_store(dram_slice, h2):
                # h2 [4c+a, 1056] -> dram rows (c a)-major = plain reshape
                nc.gpsimd.dma_start(
                    dram_slice.rearrange("c (a f) -> (c a) f", a=FOLD), h2[:])

            def load_x(d, i, t):
                nc.gpsimd.dma_start(Xb[d, i][96:128, :], xpX[i, t])
                nc.gpsimd.dma_start(Yb[d, i][:], xpY[i, t])

            def load_l1(i, t):
                # h0d rows are stored in the 66-pitch folded layout, so the
                # interior write is one contiguous span.
                nc.gpsimd.dma_start(L1b[i][0:64, hp + 1:hp + 1 + FT],
                                    h0d[i, t])

            def conv_gates(mms, bias_col, first, key):
                """mms(ps, jj, j): emit matmuls for chunk j into ps column
                block jj.  Evacuate with the unified sigmoid; run the cell;
                return h2 [128, FW] f16d (h at half scale).

                Folded tiles (F, t1h, t2, c, C2, h2) use partition layout
                p = 4*ch + a (a = spatial quarter) and 66-pitch rows, so
                every fold / h DMA has plain single-partition-dim APs."""
                S = spool.tile([128, hw], f16d, tag="S", name="S")
                F = fpool.tile([128, 4 * FW], f16d, tag="F", name="F")
                # zero the junk columns (cols 64,65 of every 66-pitch row)
                nc.vector.memset(
                    F[:, :].rearrange("p (g x) -> p g x", x=hp)[:, :, W:hp],
                    0.0)
                for hlf in range(2):
                    ps = psum_pool.tile([128, half_hw], f32, tag="ps",
                                        name="ps")
                    for jj in range(NCH // 2):
                        mms(ps, jj, hlf * (NCH // 2) + jj)
                    # sigmoid(scale*z + bias) for all 4 gates; the g rows use
                    # scale 2 / bias 2b (tanh(x) = 2*sigmoid(2x) - 1)
                    nc.scalar.activation(
                        S[:, half_hw * hlf:half_hw * (hlf + 1)], ps[:],
                        AF.Sigmoid,
                        bias=bias_sb[:, bias_col:bias_col + 1],
                        scale=bias_sb[:, 4:5])
                # fold: S[32q+c, (a rr x)] -> F[4c+a, (q rr 66)]
                for q in range(4):
                    dst = (F[:, FW * q:FW * (q + 1)]
                           .rearrange("p (r x) -> p r x", x=hp)[:, :, 0:W])
                    nc.sync.dma_start(dst, S[32 * q:32 * q + 32, :])

                Fi, Ff = F[:, 0:FW], F[:, FW:2 * FW]
                Fo, Fg = F[:, 2 * FW:3 * FW], F[:, 3 * FW:4 * FW]
                c = cst[key]
                t1h = vpool.tile([128, FW], f16d, tag="t1h", name="t1h")
                nc.vector.scalar_tensor_tensor(
                    t1h[:], Fg, 0.5, Fi, OP.subtract, OP.mult)
                if first:
                    nc.vector.tensor_scalar_mul(c[:], t1h[:], 2.0)
                else:
                    t2 = vpool.tile([128, FW], f32, tag="t2", name="t2")
                    nc.vector.tensor_mul(t2[:], Ff, c[:])
                    nc.vector.scalar_tensor_tensor(
                        c[:], t1h[:], 2.0, t2[:], OP.mult, OP.add)
                C2 = vpool.tile([128, FW], f16d, tag="C2", name="C2")
                nc.scalar.activation(C2[:], c[:], AF.Sigmoid, scale=2.0)
                h2 = hpool.tile([128, FW], f16d, tag="h2", name="h2")
                nc.vector.scalar_tensor_tensor(
                    h2[:], C2[:], 0.5, Fo, OP.subtract, OP.mult)
                if debug_taps and key == ("l0", 0, 0) and first:
                    nc.gpsimd.dma_start(dbgS[:], S[:])
                    nc.gpsimd.dma_start(dbgF[:], F[:])
                    nc.gpsimd.dma_start(dbgh[:], h2[:])
                return h2

            def l0_mms(d):
                wX = wX0f_sb if d == 0 else wX0b_sb
                wY = wY0f_sb if d == 0 else wY0b_sb

                def mms(ps, jj, j, i=None, _wX=wX, _wY=wY, _d=d):
                    X, Y = Xb[_d, mms.img], Yb[_d, mms.img]
                    pscol = ps[0:128, chunk * jj:chunk * (jj + 1)]
                    for p in range(3):
                        nc.tensor.matmul(
                            pscol, _wX[0:128, 128 * p:128 * (p + 1)],
                            rhs_slice(X, 128, p, 0, j),
                            start=(p == 0), stop=False)
                    nc.tensor.matmul(
                        pscol, _wY[0:48, 0:128],
                        rhs_slice(Y, 48, 0, 0, j),
                        start=False, stop=True)
                return mms

            l0mm = [l0_mms(0), l0_mms(1)]

            def l1_mms(w_sb, buf):
                def mms(ps, jj, j):
                    pscol = ps[0:128, chunk * jj:chunk * (jj + 1)]
                    for tap in range(9):
                        dy, dx = tap // 3, tap % 3
                        nc.tensor.matmul(
                            pscol, w_sb[0:96, 128 * tap:128 * (tap + 1)],
                            rhs_slice(buf, 96, dy, dx, j),
                            start=(tap == 0), stop=(tap == 8))
                return mms

            # ---- layer 0: 4 interleaved streams (dir x img) ----------------
            for d in range(2):
                for i in range(bl):
                    load_x(d, i, 0 if d == 0 else t_steps - 1)

            for k in range(t_steps):
                for d in range(2):
                    t = k if d == 0 else t_steps - 1 - k
                    t_nxt = k + 1 if d == 0 else t_steps - 2 - k
                    for i in range(bl):
                        mm = l0mm[d]
                        mm.img = i
                        h2 = conv_gates(mm, d, first=(k == 0),
                                        key=("l0", d, i))
                        hslice = h0d[i, t, HID * d:HID * (d + 1), :]
                        h_store(hslice, h2)
                        if k + 1 < t_steps:
                            for s in range(3):
                                nc.sync.dma_start(
                                    h_dst(Xb[d, i], 32 * s, s), hslice)
                            load_x(d, i, t_nxt)

            # ---- layer 1 forward -------------------------------------------
            # re-zero the reused fc buffers (stale layer-0 h copies would
            # leak into pad columns the fc conv reads)
            for i in range(bl):
                nc.vector.memset(fcX[i][:], 0.0)
                nc.vector.memset(fcY[i][0:64, :], 0.0)
            for i in range(bl):
                load_l1(i, 0)
            for k in range(t_steps):
                for i in range(bl):
                    last = k == t_steps - 1
                    h2 = conv_gates(l1_mms(w1f_sb, L1b[i]), 2,
                                    first=(k == 0), key=("l1", i))
                    if last:
                        h_store(fcd[i, 0], h2)
                        nc.sync.dma_start(h_dst(fcX[i], 0, 0), fcd[i, 0])
                        nc.sync.dma_start(h_dst(fcX[i], 64, 1), fcd[i, 0])
                        nc.sync.dma_start(h_dst(fcY[i], 0, 2), fcd[i, 0])
                    else:
                        h_store(h1d[i], h2)
                        nc.sync.dma_start(h_dst(L1b[i], 64, 0), h1d[i])
                        load_l1(i, k + 1)

            # ---- layer 1 backward: single step on t = T-1, h = c = 0 -------
            for i in range(bl):
                nc.vector.memset(L1b[i][64:96, :], 0.0)
                load_l1(i, t_steps - 1)
                h2 = conv_gates(l1_mms(w1b_sb, L1b[i]), 3, first=True,
                                key=("l1", i))
                h_store(fcd[i, 1], h2)
                nc.sync.dma_start(h_dst(fcX[i], 32, 0), fcd[i, 1])
                nc.sync.dma_start(h_dst(fcX[i], 96, 1), fcd[i, 1])
                nc.sync.dma_start(h_dst(fcY[i], 32, 2), fcd[i, 1])

            # ---- final conv ------------------------------------------------
            for i in range(bl):
                for hlf in range(2):
                    ps = psum_pool.tile([128, half_hw], f32, tag="ps",
                                        name="ps")
                    for jj in range(NCH // 2):
                        j = hlf * (NCH // 2) + jj
                        pscol = ps[0:64, chunk * jj:chunk * (jj + 1)]
                        for p in range(3):
                            nc.tensor.matmul(
                                pscol, wfX_sb[0:128, 64 * p:64 * (p + 1)],
                                rhs_slice(fcX[i], 128, p, 0, j),
                                start=(p == 0), stop=False)
                        for p in range(3):
                            nc.tensor.matmul(
                                pscol, wfY_sb[0:64, 64 * p:64 * (p + 1)],
                                rhs_slice(fcY[i], 64, p, 0, j),
                                start=False, stop=(p == 2))
                    Z = spool.tile([64, half_hw], f32, tag="Z", name="Z")
                    nc.scalar.activation(
                        Z[:], ps[0:64, :], AF.Identity,
                        bias=bias_sb[0:64, 5:6])
                    nc.sync.dma_start(
                        out[i][:, half_hw * hlf:half_hw * (hlf + 1)], Z[:])

    nc.compile()
    return nc


def _get_program(key=(BL, T)):
    if key not in _PROGRAM_CACHE:
        _PROGRAM_CACHE[key] = _build_program(*key)
    return _PROGRAM_CACHE[key]


# ----------------------------------------------------------------------------
# Entry point
# ----------------------------------------------------------------------------

def _prep_in_maps(x_audio, midi_notes, enc_w1, enc_b1, enc_w2, enc_b2,
                  w0f, b0f, w0b, b0b, w1f, b1f, w1b, b1b, fc_w, fc_b,
                  n_cores=NCORES):
    bsz = x_audio.shape[0]
    bl = bsz // n_cores
    pe = _note_encoder_pe(midi_notes, enc_w1, enc_b1, enc_w2, enc_b2)
    x = (np.asarray(x_audio, np.float32)
         + pe.reshape(x_audio.shape)).astype(np.float32)
    x = x.reshape(bsz * T, C, H, W)

    # padded/shifted copies of x for the packed layer-0 conv
    xpX = np.concatenate([_pad_shift(x, 0, 0), _pad_shift(x, 0, 1)],
                         axis=1).reshape(bsz, T, 32, padn).astype(f16)
    xpY = np.concatenate([_pad_shift(x, 0, 2), _pad_shift(x, 1, 2),
                          _pad_shift(x, 2, 2)],
                         axis=1).reshape(bsz, T, 48, padn).astype(f16)

    # weights: reference cin order for layer 0 is [x(16); h(32)]
    wX0f_t, wY0f_t = _pack_l0_weights(np.asarray(w0f, np.float32))
    wX0b_t, wY0b_t = _pack_l0_weights(np.asarray(w0b, np.float32))
    w1f_t = _pack_l1_weights(np.asarray(w1f, np.float32))
    w1b_t = _pack_l1_weights(np.asarray(w1b, np.float32))
    wfX_t, wfY_t = _pack_fc_weights(np.asarray(fc_w, np.float32))

    biases = np.zeros((128, 6), np.float32)
    for col, b in enumerate((b0f, b0b, b1f, b1b)):
        bv = np.asarray(b, np.float32).copy()
        bv[96:128] *= 2.0          # g rows: sigmoid(2z + 2b)
        biases[:, col] = bv
    biases[0:96, 4] = 1.0          # activation scale: i,f,o rows
    biases[96:128, 4] = 2.0        # g rows
    biases[0:64, 5] = np.asarray(fc_b, np.float32)

    in_maps = []
    for core in range(n_cores):
        sl = slice(core * bl, (core + 1) * bl)
        in_maps.append({
            "xpX": np.ascontiguousarray(xpX[sl]),
            "xpY": np.ascontiguousarray(xpY[sl]),
            "wX0f": wX0f_t, "wY0f": wY0f_t,
            "wX0b": wX0b_t, "wY0b": wY0b_t,
            "w1f": w1f_t, "w1b": w1b_t,
            "wfX": wfX_t, "wfY": wfY_t,
            "biases": biases,
        })
    return in_maps


def kernel(x_audio, midi_notes, enc_w1, enc_b1, enc_w2, enc_b2,
           w0f, b0f, w0b, b0b, w1f, b1f, w1b, b1b, fc_w, fc_b):
    from concourse.bass_utils import run_bass_kernel_spmd

    nc = _get_program()
    in_maps = _prep_in_maps(
        x_audio, midi_notes, enc_w1, enc_b1, enc_w2, enc_b2,
        w0f, b0f, w0b, b0b, w1f, b1f, w1b, b1b, fc_w, fc_b,
    )
    res = run_bass_kernel_spmd(nc, in_maps, list(range(NCORES)))
    outs = [r["out"].reshape(BL, LAT, H, W) for r in res.results]
    return np.concatenate(outs, axis=0).astype(np.float32)


# revision 27
# speedup vs baseline: 1.6507x; 1.2863x over previous
"""ConvLSTM latent-cell kernel for 8x Trainium2 NeuronCores.

Model (matches reference):
  x = x_audio + pe(midi_notes)                       [B,T,16,64,64]
  layer0 = bidirectional ConvLSTM(16 -> 32), 3x3 SAME
  layer1 = bidirectional ConvLSTM(64 -> 32), 3x3 SAME
  out    = conv3x3(layer1_out[T-1], 64 -> 64) + bias [B,64,64,64]

Sharding: data-parallel over batch, B=16 -> 2 images per core, weights
replicated, no collectives.  Only h[-1] of layer 1 is consumed, so the
layer-1 backward scan runs a single step.

Per-core program highlights:
  * conv = shifted matmuls over a zero-padded [*, 66*66] bf16 buffer.
    Layer 0 packs (channel, tap) pairs into 4 passes/step (3x K=128 +
    1x K=48) using column/row-shifted input copies; layer 1 runs the
    plain 9 passes of K=96.
  * all 4 gates are evacuated from PSUM by a single wide sigmoid
    activation per PSUM half using tanh(x) = 2*sigmoid(2x) - 1 with a
    per-partition scale vector (2 on the g rows); h is kept at HALF
    scale ((sig(2c)-0.5)*sig(o)) which is compensated by doubling the
    h-input rows of downstream conv weights.
  * gate re-tiling ("fold") into the [128 = 4 spatial blocks x 32ch,
    1024] layout is ONE dma per PSUM half via partition<->free
    rearrange; h write-back / h0 store / x loads are one dma each.
  * forward and backward layer-0 streams are interleaved (4 independent
    recurrences) to keep the PE warm and hide the per-step gate tail.
"""

import numpy as np
import ml_dtypes

B, T, C, H, W = 16, 8, 16, 64, 64
HID, LAT, KS = 32, 64, 3
NCORES = 8
BL = B // NCORES

hp = H + 2                 # padded row width
padn = hp * hp             # padded image size
hw = H * W
FOLD = 4
half_hw = hw // 2
NCH = 8                    # matmul free-dim chunks over hw (512 each)
chunk = hw // NCH
rows_per_chunk = H // NCH
# folded tiles keep a 66-column row pitch (2 "junk" columns per 64-pixel
# row that compute to exactly 0 through the gate chain) so that the
# h write-back into the padded conv buffers is a single contiguous-span
# DMA per shifted copy.
FW = 16 * hp               # 1056: one fold block (16 rows x 66)
FT = FOLD * FW             # 4224: full folded free size

f16 = np.float16


# ----------------------------------------------------------------------------
# Host-side preprocessing
# ----------------------------------------------------------------------------

def _note_encoder_pe(midi_notes, enc_w1, enc_b1, enc_w2, enc_b2):
    notes = (-0.5 + midi_notes / np.float32(64.0)).astype(np.float32)
    a = np.maximum(notes @ enc_w1 + enc_b1, 0.0).astype(np.float32)
    pe = np.tanh(a @ enc_w2 + enc_b2).astype(np.float32)
    return pe  # [B, T, C*H*W]


def _pad_shift(x, dy, dx):
    """x: [N, ch, H, W] -> padded [N, ch, hp, hp] such that padded (r, c)
    holds x[r-1+dy, c-1+dx] (zero outside)."""
    n, ch = x.shape[0], x.shape[1]
    out = np.zeros((n, ch, hp, hp), np.float32)
    r0, r1 = max(0, 1 - dy), min(hp, H + 1 - dy)
    c0, c1 = max(0, 1 - dx), min(hp, W + 1 - dx)
    out[:, :, r0:r1, c0:c1] = x[:, :, r0 - 1 + dy:r1 - 1 + dy,
                                c0 - 1 + dx:c1 - 1 + dx]
    return out


def _pack_l0_weights(w):
    """w: [128, 48, 3, 3] with cin = [x(16); h(32)].
    Returns WX [128, 3*128], WY [48, 128].
    X partition layout: [h@(0,0); h@(0,1); h@(0,2); x@(0,0); x@(0,1)].
    h rows are scaled 2x (h is stored at half scale)."""
    wx = w[:, 0:C]          # [128, 16, 3, 3]
    wh = w[:, C:] * 2.0     # [128, 32, 3, 3]
    WX = np.zeros((3, 128, 128), np.float32)
    for p in range(3):
        WX[p, 0:32] = wh[:, :, p, 0].T
        WX[p, 32:64] = wh[:, :, p, 1].T
        WX[p, 64:96] = wh[:, :, p, 2].T
        WX[p, 96:112] = wx[:, :, p, 0].T
        WX[p, 112:128] = wx[:, :, p, 1].T
    WY = np.zeros((48, 128), np.float32)
    for p in range(3):
        WY[16 * p:16 * (p + 1)] = wx[:, :, p, 2].T
    return (WX.transpose(1, 0, 2).reshape(128, 3 * 128).astype(f16),
            WY.astype(f16))


def _pack_l1_weights(w):
    """w: [128, 96, 3, 3], cin = [h0f; h0b; h1] (all half-scale -> 2x).
    Returns [96, 9*128], col block (3*dy+dx)."""
    ws = (w * 2.0).transpose(1, 2, 3, 0).reshape(96, 9 * 128)
    return np.ascontiguousarray(ws).astype(f16)


def _pack_fc_weights(w):
    """w: [64, 64, 3, 3], cin = [h1f(32); h1b(32)] half-scale -> 2x.
    fcX layout [h1f@(0,0); h1b@(0,0); h1f@(0,1); h1b@(0,1)],
    fcY layout [h1f@(0,2); h1b@(0,2)].
    Returns WX [128, 3*64], WY [64, 3*64]."""
    w2 = w * 2.0
    WX = np.zeros((3, 128, 64), np.float32)
    WY = np.zeros((3, 64, 64), np.float32)
    for p in range(3):
        WX[p, 0:64] = w2[:, :, p, 0].T
        WX[p, 64:128] = w2[:, :, p, 1].T
        WY[p] = w2[:, :, p, 2].T
    return (WX.transpose(1, 0, 2).reshape(128, 3 * 64).astype(f16),
            WY.transpose(1, 0, 2).reshape(64, 3 * 64).astype(f16))


# ----------------------------------------------------------------------------
# Device program
# ----------------------------------------------------------------------------

_PROGRAM_CACHE = {}


def _build_program(bl=BL, t_steps=T, debug_taps=False):
    import contextlib
    import concourse.bass as bass
    import concourse.tile as tile
    from concourse import bacc, mybir

    f32 = mybir.dt.float32
    f16d = mybir.dt.float16
    AF = mybir.ActivationFunctionType
    OP = mybir.AluOpType

    nc = bacc.Bacc("TRN2", target_bir_lowering=False, debug=False,
                   num_devices=1)

    xpX = nc.dram_tensor("xpX", [bl, t_steps, 32, padn], f16d,
                         kind="ExternalInput")
    xpY = nc.dram_tensor("xpY", [bl, t_steps, 48, padn], f16d,
                         kind="ExternalInput")
    wX0f = nc.dram_tensor("wX0f", [128, 384], f16d, kind="ExternalInput")
    wY0f = nc.dram_tensor("wY0f", [48, 128], f16d, kind="ExternalInput")
    wX0b = nc.dram_tensor("wX0b", [128, 384], f16d, kind="ExternalInput")
    wY0b = nc.dram_tensor("wY0b", [48, 128], f16d, kind="ExternalInput")
    w1f = nc.dram_tensor("w1f", [96, 1152], f16d, kind="ExternalInput")
    w1b = nc.dram_tensor("w1b", [96, 1152], f16d, kind="ExternalInput")
    wfX = nc.dram_tensor("wfX", [128, 192], f16d, kind="ExternalInput")
    wfY = nc.dram_tensor("wfY", [64, 192], f16d, kind="ExternalInput")
    biases = nc.dram_tensor("biases", [128, 6], f32, kind="ExternalInput")
    out = nc.dram_tensor("out", [bl, LAT, hw], f32, kind="ExternalOutput")
    if debug_taps:
        h0d = nc.dram_tensor("h0d", [bl, t_steps, 2 * HID, FT], f16d,
                             kind="ExternalOutput")
    else:
        h0d = nc.dram_tensor("h0d", [bl, t_steps, 2 * HID, FT], f16d)
    h1d = nc.dram_tensor("h1d", [bl, 32, FT], f16d)
    fcd = nc.dram_tensor("fcd", [bl, 2, 32, FT], f16d)
    if debug_taps:
        dbgS = nc.dram_tensor("dbgS", [128, FT], f16d, kind="ExternalOutput")
        dbgF = nc.dram_tensor("dbgF", [128, FT], f16d, kind="ExternalOutput")
        dbgh = nc.dram_tensor("dbgh", [128, FW], f16d, kind="ExternalOutput")

    with tile.TileContext(nc) as tc:
        with contextlib.ExitStack() as ctx:
            persist = ctx.enter_context(tc.tile_pool(name="persist", bufs=1))
            spool = ctx.enter_context(tc.tile_pool(name="spool", bufs=2))
            fpool = ctx.enter_context(tc.tile_pool(name="fpool", bufs=2))
            vpool = ctx.enter_context(tc.tile_pool(name="vpool", bufs=2))
            hpool = ctx.enter_context(tc.tile_pool(name="hpool", bufs=3))
            psum_pool = ctx.enter_context(
                tc.tile_pool(name="ps", bufs=2, space="PSUM"))

            # ---- persistent tiles ------------------------------------------
            wX0f_sb = persist.tile([128, 384], f16d, tag="wX0f")
            wY0f_sb = persist.tile([48, 128], f16d, tag="wY0f")
            wX0b_sb = persist.tile([128, 384], f16d, tag="wX0b")
            wY0b_sb = persist.tile([48, 128], f16d, tag="wY0b")
            w1f_sb = persist.tile([96, 1152], f16d, tag="w1f")
            w1b_sb = persist.tile([96, 1152], f16d, tag="w1b")
            wfX_sb = persist.tile([128, 192], f16d, tag="wfX")
            wfY_sb = persist.tile([64, 192], f16d, tag="wfY")
            bias_sb = persist.tile([128, 6], f32, tag="bias")
            for sb, dr in ((wX0f_sb, wX0f), (wY0f_sb, wY0f),
                           (wX0b_sb, wX0b), (wY0b_sb, wY0b),
                           (w1f_sb, w1f), (w1b_sb, w1b),
                           (wfX_sb, wfX), (wfY_sb, wfY), (bias_sb, biases)):
                nc.sync.dma_start(sb[:], dr[:])

            # conv input buffers: per (dir, img) for L0; per img for L1/FC
            Xb = {}
            Yb = {}
            for d in range(2):
                for i in range(bl):
                    Xb[d, i] = persist.tile([128, padn], f16d,
                                            tag=f"X{d}{i}", name=f"X{d}{i}")
                    Yb[d, i] = persist.tile([48, padn], f16d,
                                            tag=f"Y{d}{i}", name=f"Y{d}{i}")
            L1b = [persist.tile([96, padn], f16d, tag=f"L1b{i}",
                                name=f"L1b{i}") for i in range(bl)]
            # fc conv buffers reuse the (done-by-then) layer-0 X tiles;
            # they are re-zeroed after layer 0 finishes.
            fcX = [Xb[0, i] for i in range(bl)]
            fcY = [Xb[1, i] for i in range(bl)]
            # c states: 4 L0 streams; L1 reuses the d=0 tiles
            cst = {}
            for d in range(2):
                for i in range(bl):
                    cst["l0", d, i] = persist.tile(
                        [128, FW], f32, tag=f"c0{d}{i}", name=f"c0{d}{i}")
            for i in range(bl):
                cst["l1", i] = cst["l0", 0, i]

            # memsets (pads; h rows).  x rows of X / all of Y are fully
            # rewritten by the padded host images every step.
            for d in range(2):
                for i in range(bl):
                    nc.vector.memset(Xb[d, i][0:96, :], 0.0)
            for i in range(bl):
                nc.gpsimd.memset(L1b[i][:], 0.0)
                nc.gpsimd.memset(fcX[i][:], 0.0)
                nc.gpsimd.memset(fcY[i][:], 0.0)

            # ---- helpers ---------------------------------------------------
            def rhs_slice(buf, cin, dy, dx, j):
                r0 = rows_per_chunk * j + dy
                return (buf[0:cin, :]
                        .rearrange("p (r c) -> p r c", c=hp)
                        [:, r0:r0 + rows_per_chunk, dx:dx + W])

            def h_dst(buf, pbase, shift):
                start = (hp + 1) - shift
                return buf[pbase:pbase + 32, start:start + FT]

            def h_store(dram_slice, h2):
                # h2 [4c+a, 1056] -> dram rows (c a)-major = plain reshape
                nc.gpsimd.dma_start(
                    dram_slice.rearrange("c (a f) -> (c a) f", a=FOLD), h2[:])

            def load_x(d, i, t):
                nc.gpsimd.dma_start(Xb[d, i][96:128, :], xpX[i, t])
                nc.gpsimd.dma_start(Yb[d, i][:], xpY[i, t])

            def load_l1(i, t):
                # h0d rows are stored in the 66-pitch folded layout, so the
                # interior write is one contiguous span.
                nc.gpsimd.dma_start(L1b[i][0:64, hp + 1:hp + 1 + FT],
                                    h0d[i, t])

            def conv_gates(mms, bias_col, first, key):
                """mms(ps, jj, j): emit matmuls for chunk j into ps column
                block jj.  Evacuate with the unified sigmoid; run the cell;
                return h2 [128, FW] f16d (h at half scale).

                Folded tiles (F, t1h, t2, c, C2, h2) use partition layout
                p = 4*ch + a (a = spatial quarter) and 66-pitch rows with
                zeroed junk columns, so every fold / h DMA is a contiguous
                plain AP on both sides (few, large descriptors)."""
                S = spool.tile([128, FT], f16d, tag="S", name="S")
                F = fpool.tile([128, 4 * FW], f16d, tag="F", name="F")
                # zero the junk columns (cols 64,65 of every 66-pitch row)
                nc.vector.memset(
                    S[:, :].rearrange("p (g x) -> p g x", x=hp)[:, :, W:hp],
                    0.0)
                for hlf in range(2):
                    ps = psum_pool.tile([128, half_hw], f32, tag="ps",
                                        name="ps")
                    for jj in range(NCH // 2):
                        mms(ps, jj, hlf * (NCH // 2) + jj)
                    # sigmoid(scale*z + bias) for all 4 gates; the g rows use
                    # scale 2 / bias 2b (tanh(x) = 2*sigmoid(2x) - 1)
                    dstS = (S[:, 2 * FW * hlf:2 * FW * (hlf + 1)]
                            .rearrange("p (g x) -> p g x", x=hp)[:, :, 0:W])
                    nc.scalar.activation(
                        dstS, ps[:].rearrange("p (g x) -> p g x", x=W),
                        AF.Sigmoid,
                        bias=bias_sb[:, bias_col:bias_col + 1],
                        scale=bias_sb[:, 4:5])
                # fold: S[32q+c, (a rr 66)] -> F[4c+a, (q rr 66)]; both
                # sides fully contiguous
                for q in range(4):
                    nc.sync.dma_start(F[:, FW * q:FW * (q + 1)],
                                      S[32 * q:32 * q + 32, :])

                Fi, Ff = F[:, 0:FW], F[:, FW:2 * FW]
                Fo, Fg = F[:, 2 * FW:3 * FW], F[:, 3 * FW:4 * FW]
                c = cst[key]
                t1h = vpool.tile([128, FW], f16d, tag="t1h", name="t1h")
                nc.vector.scalar_tensor_tensor(
                    t1h[:], Fg, 0.5, Fi, OP.subtract, OP.mult)
                if first:
                    nc.vector.tensor_scalar_mul(c[:], t1h[:], 2.0)
                else:
                    t2 = vpool.tile([128, FW], f32, tag="t2", name="t2")
                    nc.vector.tensor_mul(t2[:], Ff, c[:])
                    nc.vector.scalar_tensor_tensor(
                        c[:], t1h[:], 2.0, t2[:], OP.mult, OP.add)
                C2 = vpool.tile([128, FW], f16d, tag="C2", name="C2")
                nc.scalar.activation(C2[:], c[:], AF.Sigmoid, scale=2.0)
                h2 = hpool.tile([128, FW], f16d, tag="h2", name="h2")
                nc.vector.scalar_tensor_tensor(
                    h2[:], C2[:], 0.5, Fo, OP.subtract, OP.mult)
                if debug_taps and key == ("l0", 0, 0) and first:
                    nc.gpsimd.dma_start(dbgS[:], S[:])
                    nc.gpsimd.dma_start(dbgF[:], F[:])
                    nc.gpsimd.dma_start(dbgh[:], h2[:])
                return h2

            def l0_mms(d):
                wX = wX0f_sb if d == 0 else wX0b_sb
                wY = wY0f_sb if d == 0 else wY0b_sb

                def mms(ps, jj, j, i=None, _wX=wX, _wY=wY, _d=d):
                    X, Y = Xb[_d, mms.img], Yb[_d, mms.img]
                    pscol = ps[0:128, chunk * jj:chunk * (jj + 1)]
                    for p in range(3):
                        nc.tensor.matmul(
                            pscol, _wX[0:128, 128 * p:128 * (p + 1)],
                            rhs_slice(X, 128, p, 0, j),
                            start=(p == 0), stop=False)
                    nc.tensor.matmul(
                        pscol, _wY[0:48, 0:128],
                        rhs_slice(Y, 48, 0, 0, j),
                        start=False, stop=True)
                return mms

            l0mm = [l0_mms(0), l0_mms(1)]

            def l1_mms(w_sb, buf):
                def mms(ps, jj, j):
                    pscol = ps[0:128, chunk * jj:chunk * (jj + 1)]
                    for tap in range(9):
                        dy, dx = tap // 3, tap % 3
                        nc.tensor.matmul(
                            pscol, w_sb[0:96, 128 * tap:128 * (tap + 1)],
                            rhs_slice(buf, 96, dy, dx, j),
                            start=(tap == 0), stop=(tap == 8))
                return mms

            # ---- layer 0: 4 interleaved streams (dir x img) ----------------
            for d in range(2):
                for i in range(bl):
                    load_x(d, i, 0 if d == 0 else t_steps - 1)

            for k in range(t_steps):
                for d in range(2):
                    t = k if d == 0 else t_steps - 1 - k
                    t_nxt = k + 1 if d == 0 else t_steps - 2 - k
                    for i in range(bl):
                        mm = l0mm[d]
                        mm.img = i
                        h2 = conv_gates(mm, d, first=(k == 0),
                                        key=("l0", d, i))
                        hslice = h0d[i, t, HID * d:HID * (d + 1), :]
                        h_store(hslice, h2)
                        if k + 1 < t_steps:
                            for s in range(3):
                                nc.sync.dma_start(
                                    h_dst(Xb[d, i], 32 * s, s), hslice)
                            load_x(d, i, t_nxt)

            # ---- layer 1 forward -------------------------------------------
            # re-zero the reused fc buffers (stale layer-0 h copies would
            # leak into pad columns the fc conv reads)
            for i in range(bl):
                nc.vector.memset(fcX[i][:], 0.0)
                nc.vector.memset(fcY[i][0:64, :], 0.0)
            for i in range(bl):
                load_l1(i, 0)
            for k in range(t_steps):
                for i in range(bl):
                    last = k == t_steps - 1
                    h2 = conv_gates(l1_mms(w1f_sb, L1b[i]), 2,
                                    first=(k == 0), key=("l1", i))
                    if last:
                        h_store(fcd[i, 0], h2)
                        nc.sync.dma_start(h_dst(fcX[i], 0, 0), fcd[i, 0])
                        nc.sync.dma_start(h_dst(fcX[i], 64, 1), fcd[i, 0])
                        nc.sync.dma_start(h_dst(fcY[i], 0, 2), fcd[i, 0])
                    else:
                        h_store(h1d[i], h2)
                        nc.sync.dma_start(h_dst(L1b[i], 64, 0), h1d[i])
                        load_l1(i, k + 1)

            # ---- layer 1 backward: single step on t = T-1, h = c = 0 -------
            for i in range(bl):
                nc.vector.memset(L1b[i][64:96, :], 0.0)
                load_l1(i, t_steps - 1)
                h2 = conv_gates(l1_mms(w1b_sb, L1b[i]), 3, first=True,
                                key=("l1", i))
                h_store(fcd[i, 1], h2)
                nc.sync.dma_start(h_dst(fcX[i], 32, 0), fcd[i, 1])
                nc.sync.dma_start(h_dst(fcX[i], 96, 1), fcd[i, 1])
                nc.sync.dma_start(h_dst(fcY[i], 32, 2), fcd[i, 1])

            # ---- final conv ------------------------------------------------
            for i in range(bl):
                for hlf in range(2):
                    ps = psum_pool.tile([128, half_hw], f32, tag="ps",
                                        name="ps")
                    for jj in range(NCH // 2):
                        j = hlf * (NCH // 2) + jj
                        pscol = ps[0:64, chunk * jj:chunk * (jj + 1)]
                        for p in range(3):
                            nc.tensor.matmul(
                                pscol, wfX_sb[0:128, 64 * p:64 * (p + 1)],
                                rhs_slice(fcX[i], 128, p, 0, j),
                                start=(p == 0), stop=False)
                        for p in range(3):
                            nc.tensor.matmul(
                                pscol, wfY_sb[0:64, 64 * p:64 * (p + 1)],
                                rhs_slice(fcY[i], 64, p, 0, j),
                                start=False, stop=(p == 2))
                    Z = spool.tile([64, half_hw], f32, tag="Z", name="Z")
                    nc.scalar.activation(
                        Z[:], ps[0:64, :], AF.Identity,
                        bias=bias_sb[0:64, 5:6])
                    nc.sync.dma_start(
                        out[i][:, half_hw * hlf:half_hw * (hlf + 1)], Z[:])

    nc.compile()
    return nc


def _get_program(key=(BL, T)):
    if key not in _PROGRAM_CACHE:
        _PROGRAM_CACHE[key] = _build_program(*key)
    return _PROGRAM_CACHE[key]


# ----------------------------------------------------------------------------
# Entry point
# ----------------------------------------------------------------------------

def _prep_in_maps(x_audio, midi_notes, enc_w1, enc_b1, enc_w2, enc_b2,
                  w0f, b0f, w0b, b0b, w1f, b1f, w1b, b1b, fc_w, fc_b,
                  n_cores=NCORES):
    bsz = x_audio.shape[0]
    bl = bsz // n_cores
    pe = _note_encoder_pe(midi_notes, enc_w1, enc_b1, enc_w2, enc_b2)
    x = (np.asarray(x_audio, np.float32)
         + pe.reshape(x_audio.shape)).astype(np.float32)
    x = x.reshape(bsz * T, C, H, W)

    # padded/shifted copies of x for the packed layer-0 conv
    xpX = np.concatenate([_pad_shift(x, 0, 0), _pad_shift(x, 0, 1)],
                         axis=1).reshape(bsz, T, 32, padn).astype(f16)
    xpY = np.concatenate([_pad_shift(x, 0, 2), _pad_shift(x, 1, 2),
                          _pad_shift(x, 2, 2)],
                         axis=1).reshape(bsz, T, 48, padn).astype(f16)

    # weights: reference cin order for layer 0 is [x(16); h(32)]
    wX0f_t, wY0f_t = _pack_l0_weights(np.asarray(w0f, np.float32))
    wX0b_t, wY0b_t = _pack_l0_weights(np.asarray(w0b, np.float32))
    w1f_t = _pack_l1_weights(np.asarray(w1f, np.float32))
    w1b_t = _pack_l1_weights(np.asarray(w1b, np.float32))
    wfX_t, wfY_t = _pack_fc_weights(np.asarray(fc_w, np.float32))

    biases = np.zeros((128, 6), np.float32)
    for col, b in enumerate((b0f, b0b, b1f, b1b)):
        bv = np.asarray(b, np.float32).copy()
        bv[96:128] *= 2.0          # g rows: sigmoid(2z + 2b)
        biases[:, col] = bv
    biases[0:96, 4] = 1.0          # activation scale: i,f,o rows
    biases[96:128, 4] = 2.0        # g rows
    biases[0:64, 5] = np.asarray(fc_b, np.float32)

    in_maps = []
    for core in range(n_cores):
        sl = slice(core * bl, (core + 1) * bl)
        in_maps.append({
            "xpX": np.ascontiguousarray(xpX[sl]),
            "xpY": np.ascontiguousarray(xpY[sl]),
            "wX0f": wX0f_t, "wY0f": wY0f_t,
            "wX0b": wX0b_t, "wY0b": wY0b_t,
            "w1f": w1f_t, "w1b": w1b_t,
            "wfX": wfX_t, "wfY": wfY_t,
            "biases": biases,
        })
    return in_maps


def kernel(x_audio, midi_notes, enc_w1, enc_b1, enc_w2, enc_b2,
           w0f, b0f, w0b, b0b, w1f, b1f, w1b, b1b, fc_w, fc_b):
    from concourse.bass_utils import run_bass_kernel_spmd

    nc = _get_program()
    in_maps = _prep_in_maps(
        x_audio, midi_notes, enc_w1, enc_b1, enc_w2, enc_b2,
        w0f, b0f, w0b, b0b, w1f, b1f, w1b, b1b, fc_w, fc_b,
    )
    res = run_bass_kernel_spmd(nc, in_maps, list(range(NCORES)))
    outs = [r["out"].reshape(BL, LAT, H, W) for r in res.results]
    return np.concatenate(outs, axis=0).astype(np.float32)
